# revision 31
# baseline (speedup 1.0000x reference)
"""Pipelined GEMM kernel for Trainium2, 8 NeuronCores.

Computes C = A @ B + ws*(ws+1)/2 with A:(8192,256) B:(256,8192) fp32.

Sharding: 2x4 grid over (M, N). Core (mi, ni) computes the (4096, 2048)
output block from A rows [mi] and B columns [ni]. No inter-core
communication; this minimizes per-core HBM traffic vs the K-parallel
all-reduce layout (~296MB/core) or 1x8 row sharding (41MB/core).

Precision/bandwidth tradeoff: inputs are cast to fp16 on the host as part
of sharding (A^T shard 2MB, B shard 1MB per core) and the kernel writes
its C block as fp16 (16MB), upcast to fp32 on the host. fp16 rounding of
inputs and output costs ~2.4e-4 norm rel error here (K=256, N(0,1) data,
+36 offset; gate is 2e-2) and halves HBM traffic: 19MB/core vs 38MB.
At ~358 GB/s/core that is a ~53us memory roofline, balanced against the
~55us PE roofline (131072 fp16 streaming cycles @ 2.4 GHz).

Per-core kernel (Tile framework). The m-loop invariant is that nothing
PE waits on (PSUM WAR via the evicts) ever sits behind a DMA issue or a
cross-engine ordering edge:
  - Each m-tile accumulates into FOUR 1-bank PSUM tiles (one per 512-col
    j-chunk, double-buffered = all 8 banks). Separate lo/hi tiles
    because the tile framework orders cross-engine accesses of a shared
    tile - with one [128,2048] tile ACT's evict serialized behind DVE's,
    stalling PE ~1us every other m-tile (110->83us once split); the
    further 4-way split releases each quarter's WAR as soon as its own
    k1 matmul retires (~5us more on HW).
  - +const is fused into the PSUM->SBUF evictions: DVE evicts the lo
    quarters, ACT the hi quarters, concurrently, into per-engine
    8-m-tile group tiles.
  - Output DRAM is a permuted pair clo/chi[g][p][mg][1024] so a store
    group (8 m-tiles, 2MB) is one 16KB-contiguous descriptor per
    partition: one store per 8 m-tiles per ring amortizes the fixed
    DMA-issue cost that saturated the sync sequencer with per-m-tile
    stores (and is measurably more robust in slow device phases). The
    host unpermutes (transpose+reshape) while upcasting. Lo groups ride
    the sync HWDGE ring, hi groups the gpsimd SWDGE queue; DVE/ACT
    issue no stores. The final group stores pair-wise on both HWDGE
    rings to shorten the serial tail.
  - Loads: the pieces the first m-tiles need (B[:, :512], A^T[:, :1024],
    both k) ride the HWDGE rings for the first copy after the For_i
    barrier; later copies' loads all stream on SWDGE a full copy ahead,
    where they cannot queue behind stores.
  - The timing repeat loop (tc.For_i) has an all-engine barrier per
    iteration costing ~40-50us on HW, so repeat>1 unrolls `unroll` GEMM
    executions per iteration with ping-pong input buffers: copy u+1's
    loads prefetch during copy u's m-loop, hiding the load head
    everywhere except the first copy after the barrier (measured 82us
    per GEMM at unroll=2 -> ~67-70us at unroll=16).
"""

import contextlib

import numpy as np

import concourse.mybir as mybir
import concourse.tile as tile
from concourse import bacc
from concourse.bass_utils import run_bass_kernel_spmd

M, K, N = 8192, 256, 8192
NCORES = 8
RM, RN = 2, 4  # core grid over (M, N)
MS = M // RM  # 4096 rows of C per core
NS = N // RN  # 2048 cols of C per core
P = 128
MT = MS // P  # 32 m-tiles
KT = K // P  # 2 k-tiles
NCHUNK = 512  # max matmul moving free dim
NT = NS // NCHUNK  # 4 n-chunks per m-tile
BCRIT = 512  # B cols the first m-tile needs (j0 chunk)
ACRIT = 1024  # A^T cols the first 8 m-tiles need
QUAD = 4  # m-tiles per store
NQ = MT // QUAD  # 8 quad-stores per GEMM
H = NS // 2  # evict half width

F32 = mybir.dt.float32
F16 = mybir.dt.float16


def build_program(const_add: float, repeat: int = 1, loop_opts: dict | None = None,
                  psum_bufs: int = 2, opool_bufs: int = 4, unroll: int = 16,
                  hi_store: str = "gpsimd", psum_split4: bool = True,
                  k_inner: bool = True, store_m: int = 8,
                  probe_half_k: bool = False, probe_half_store: bool = False):
    """repeat>1 wraps `unroll` ping-pong copies of the GEMM in a HW loop
    of repeat//unroll iterations - used only by the timing harness (slope
    between two repeat counts cancels the ~200ms axon dispatch
    overhead)."""
    nc = bacc.Bacc("TRN2", target_bir_lowering=False, debug=False)
    at = nc.dram_tensor("at", [K, MS], F16, kind="ExternalInput")
    b = nc.dram_tensor("b", [K, NS], F16, kind="ExternalInput")
    nq = MT // store_m
    clo = nc.dram_tensor("clo", [nq, P, store_m, H], F16, kind="ExternalOutput")
    chi = nc.dram_tensor("chi", [nq, P, store_m, H], F16, kind="ExternalOutput")

    if repeat > 1:
        assert repeat % unroll == 0, (repeat, unroll)
        ncopies = unroll
    else:
        ncopies = 1

    with tile.TileContext(nc) as tc:
        with (
            tc.tile_pool(name="bpool", bufs=1) as bpool,
            tc.tile_pool(name="atpool", bufs=1) as atpool,
            tc.tile_pool(name="pslo", bufs=psum_bufs, space="PSUM") as pslo_pool,
            tc.tile_pool(name="pshi", bufs=psum_bufs, space="PSUM") as pshi_pool,
            tc.tile_pool(name="opool", bufs=opool_bufs) as opool,
            tc.For_i(0, repeat // ncopies, 1, **(loop_opts or {}))
            if repeat > ncopies else contextlib.nullcontext(),
        ):
            nsets = min(2, ncopies)
            b_sb = [
                [[bpool.tile([P, NS - BCRIT if piece else BCRIT], F16,
                             name=f"b{u}k{k}p{piece}", tag=f"b{u}k{k}p{piece}")
                  for piece in range(2)]
                 for k in range(KT)]
                for u in range(nsets)
            ]
            at_sb = [
                [[atpool.tile([P, MS - ACRIT if piece else ACRIT], F16,
                              name=f"at{u}k{k}p{piece}", tag=f"at{u}k{k}p{piece}")
                  for piece in range(2)]
                 for k in range(KT)]
                for u in range(nsets)
            ]

            def load_set(u, head=False):
                """Critical pieces first.  For the first copy after the
                For_i barrier they ride the idle HWDGE rings (two
                parallel ~0.6us-latency queues beat the ~1us/DMA SWDGE
                issue serialization); for later copies the SWDGE queue
                issues them a full copy ahead of when they are needed,
                whereas on the HWDGE rings they would queue behind the
                previous copy's stores."""
                crit = [nc.sync, nc.scalar] if head else [nc.gpsimd, nc.gpsimd]
                for k in range(KT):
                    crit[k].dma_start(b_sb[u][k][0][:],
                                      b[k * P:(k + 1) * P, :BCRIT])
                for k in range(KT):
                    crit[k].dma_start(at_sb[u][k][0][:],
                                      at[k * P:(k + 1) * P, :ACRIT])
                for k in range(KT):
                    nc.gpsimd.dma_start(b_sb[u][k][1][:],
                                        b[k * P:(k + 1) * P, BCRIT:])
                for k in range(KT):
                    nc.gpsimd.dma_start(at_sb[u][k][1][:],
                                        at[k * P:(k + 1) * P, ACRIT:])

            def b_slice(u, k, j):
                lo = j * NCHUNK
                if lo < BCRIT:
                    return b_sb[u][k][0][:, lo:lo + NCHUNK]
                return b_sb[u][k][1][:, lo - BCRIT:lo - BCRIT + NCHUNK]

            def at_slice(u, k, m):
                lo = m * P
                if lo < ACRIT:
                    return at_sb[u][k][0][:, lo:lo + P]
                return at_sb[u][k][1][:, lo - ACRIT:lo - ACRIT + P]

            def mloop(u, tail):
                for q in range(nq):
                    olo = opool.tile([P, store_m * H], F16, name="olo", tag="olo")
                    ohi = opool.tile([P, store_m * H], F16, name="ohi", tag="ohi")
                    split_last = tail and q == nq - 1
                    for mq in range(store_m):
                        m = q * store_m + mq
                        if psum_split4:
                            # one 1-bank PSUM tile per j-chunk: each
                            # quarter's WAR releases as soon as its own
                            # k1 matmul retires, giving the PSUM->evict
                            # ->matmul chain an extra ~0.6us of slack
                            pt = [pslo_pool.tile([P, NCHUNK], F32,
                                                 name=f"p{j}", tag=f"p{j}")
                                  if j < NT // 2 else
                                  pshi_pool.tile([P, NCHUNK], F32,
                                                 name=f"p{j}", tag=f"p{j}")
                                  for j in range(NT)]
                        else:
                            pl = pslo_pool.tile([P, H], F32, name="pl", tag="pl")
                            ph = pshi_pool.tile([P, H], F32, name="ph", tag="ph")
                        kt = 1 if probe_half_k else KT
                        if psum_split4 and k_inner:
                            # k-inner: each quarter's accumulation
                            # retires as early as possible, spreading
                            # the evicts (and their PSUM WAR releases)
                            # across the m-tile instead of bunching
                            # them at its end.  LDWEIGHTS alternates
                            # every matmul but FWL + the PE's pull-
                            # ahead weight buffer hide it under the
                            # 512-col streams.
                            for j in range(NT):
                                for k in range(kt):
                                    nc.tensor.matmul(
                                        pt[j][:],
                                        at_slice(u, k, m),
                                        b_slice(u, k, j),
                                        start=(k == 0),
                                        stop=(k == kt - 1),
                                    )
                        else:
                            for k in range(kt):
                                w = at_slice(u, k, m)
                                for j in range(NT):
                                    if psum_split4:
                                        dst = pt[j][:]
                                    else:
                                        jj = j % (NT // 2)
                                        dst = (pl if j < NT // 2 else ph)[
                                            :, jj * NCHUNK:(jj + 1) * NCHUNK]
                                    nc.tensor.matmul(
                                        dst,
                                        w,
                                        b_slice(u, k, j),
                                        start=(k == 0),
                                        stop=(k == kt - 1),
                                    )
                        if psum_split4:
                            for j in range(NT // 2):
                                nc.vector.tensor_scalar_add(
                                    olo[:, mq * H + j * NCHUNK:
                                        mq * H + (j + 1) * NCHUNK],
                                    pt[j][:], const_add)
                            for j in range(NT // 2, NT):
                                jj = j - NT // 2
                                nc.scalar.activation(
                                    ohi[:, mq * H + jj * NCHUNK:
                                        mq * H + (jj + 1) * NCHUNK],
                                    pt[j][:],
                                    mybir.ActivationFunctionType.Copy,
                                    bias=const_add,
                                )
                        else:
                            nc.vector.tensor_scalar_add(
                                olo[:, mq * H:(mq + 1) * H], pl[:], const_add)
                            nc.scalar.activation(
                                ohi[:, mq * H:(mq + 1) * H], ph[:],
                                mybir.ActivationFunctionType.Copy,
                                bias=const_add,
                            )
                        if split_last and mq % 2 == 1:
                            # the body's very last quad stores pair-wise
                            # on both HWDGE rings (scalar is idle by
                            # now) so the serial tail is one evict plus
                            # a 512KB store, not a 1MB quad store
                            pr = slice(mq - 1, mq + 1)
                            cw = slice((mq - 1) * H, (mq + 1) * H)
                            nc.sync.dma_start(clo[q][:, pr, :], olo[:, cw])
                            nc.scalar.dma_start(chi[q][:, pr, :], ohi[:, cw])
                    if not split_last:
                        nc.sync.dma_start(clo[q], olo[:])
                        if not probe_half_store:
                            getattr(nc, hi_store).dma_start(chi[q], ohi[:])

            load_set(0, head=True)
            for u in range(1, ncopies):
                load_set(u % nsets)
                mloop((u - 1) % nsets, tail=False)
            mloop((ncopies - 1) % nsets, tail=True)

    nc.compile()
    return nc


_CACHE = {}


def _get_program(const_add: float):
    key = const_add
    if key not in _CACHE:
        _CACHE[key] = build_program(const_add)
    return _CACHE[key]


def make_in_maps(A, B):
    """2x4 (M, N) grid; A shards staged K-major; fp16 staging."""
    maps = []
    for i in range(NCORES):
        mi, ni = divmod(i, RN)
        maps.append({
            "at": np.ascontiguousarray(
                A[mi * MS:(mi + 1) * MS].T.astype(np.float16)),
            "b": np.ascontiguousarray(
                B[:, ni * NS:(ni + 1) * NS].astype(np.float16)),
        })
    return maps


def unpermute(clo_core, chi_core):
    """[NQ, P, QUAD, H] fp16 pair -> [MS, NS] fp32 C block."""
    lo = np.asarray(clo_core).transpose(0, 2, 1, 3).reshape(MS, H)
    hi = np.asarray(chi_core).transpose(0, 2, 1, 3).reshape(MS, H)
    return np.concatenate([lo, hi], axis=1).astype(np.float32)


def assemble(results):
    rows = []
    for mi in range(RM):
        rows.append(np.concatenate(
            [unpermute(results[mi * RN + ni]["clo"],
                       results[mi * RN + ni]["chi"]) for ni in range(RN)],
            axis=1))
    return np.concatenate(rows, axis=0)


def run(A, B, world_size, trace=False, **spmd_kwargs):
    A = np.ascontiguousarray(np.asarray(A, dtype=np.float32))
    B = np.ascontiguousarray(np.asarray(B, dtype=np.float32))
    ws = int(world_size)
    const_add = float(ws * (ws + 1) / 2)
    assert A.shape == (M, K) and B.shape == (K, N)

    nc = _get_program(const_add)
    res = run_bass_kernel_spmd(
        nc, make_in_maps(A, B), list(range(NCORES)), trace=trace, **spmd_kwargs
    )
    return assemble(res.results), res


def kernel(A, B, world_size, **_unused):
    out, _ = run(A, B, world_size, trace=False)
    return out
